# revision 13
# baseline (speedup 1.0000x reference)
"""Trainium2 Bass kernel for DirectionalConvLayer.

Problem: 4 directional 3-tap convs over [256, 256, 15, 15] fp32 images, one
input per direction (horizontal / vertical / main-diagonal / anti-diagonal
taps), shared weight [256, 256, 3] and bias [256].

Strategy: every direction is a 1-D 3-tap conv along its set of lines
(rows / columns / diagonals / anti-diagonals) with a dense 256x256 channel
mix per tap. On the host, ALL lines of ALL four inputs are packed
back-to-back (no separators) into one flat stream, split across 8 cores at
line boundaries. The device kernel is direction-agnostic: a pure 3-tap conv
along the flat axis. The conv contaminates the two outputs at every line
junction with one known term each; the host subtracts those (two batched
matmuls) during unpacking.

PE-work reduction via partial Winograd F(2,2): for an output pair
(y_e, y_o) at stream cols (2j, 2j+1), with e[j] = x[2j], o[j] = x[2j+1]:
  y_e = W0 o[j-1] + W1 e[j] + W2 o[j]
  y_o = W0 e[j]   + W1 o[j] + W2 e[j+1]
Winograd F(2,2) on the (W1, W2) 2-tap part shares the midpoint product
M = (W1+W2) o[j]:
  y_e = [W1 (e[j]-o[j])     + W0 o[j-1]] + M   = A + M
  y_o = [W2 (e[j+1]-o[j])   + W0 e[j]  ] + M   = B + M
=> 5 matmul products per 2 output columns instead of the naive 6 (PE time
x5/6), at the cost of 2 cheap fp16 difference streams (DVE + GPSIMD), one
PSUM->SBUF copy of M per cout-half (ScalarE), and PSUM+SBUF adds (DVE).
All transform work hides under the PE.

Transfers and matmul operands are float16 (10-bit mantissa); PSUM
accumulates in fp32. Host packs even/odd deinterleaved streams (guard
offsets chosen so the d1 subtraction is 4B-aligned -> DVE 2x mode); host
adds bias and fixes line junctions during unpacking.
"""
from contextlib import ExitStack

import numpy as np

import concourse.bass as bass
import concourse.tile as tile
from concourse import mybir
from concourse.bass_utils import run_bass_kernel_spmd

P = 128
H = W = 15
NCORE = 8
CORE_COLS = 28808          # stream cols per core (must be even)
NP = CORE_COLS // 2        # 14404 output pairs per core
TOT = NP + 2               # packed stream length incl. guards

MM_DT = mybir.dt.float16
MM_NP = np.float16

# per-block pair counts: small prologue for fast PE start, small tail
# (all even: required for 4B alignment of the d1 subtraction views)
UNITS = [128, 128, 256] + [512] * 26 + [256, 160, 100, 64]
assert sum(UNITS) == NP and all(u % 2 == 0 for u in UNITS)


def _build_lines(d):
    if d == 0:
        return [[(i, j) for j in range(W)] for i in range(H)]
    if d == 1:
        return [[(i, j) for i in range(H)] for j in range(W)]
    if d == 2:
        return [
            [(i, i - k) for i in range(max(0, k), min(H, H + k))]
            for k in range(-(W - 1), W)
        ]
    return [
        [(i, s - i) for i in range(max(0, s - (W - 1)), min(H, s + 1))]
        for s in range(H + W - 1)
    ]


def _build_stream_map():
    """Greedy-pack every (direction, image, line) into NCORE x CORE_COLS,
    back-to-back with NO separators. colmap[d, b, i*W+j] = core * CORE_COLS
    + local_col. At every line-to-line junction the device conv contaminates
    the two adjacent outputs (tap w2 of the left line's last cell reads the
    right line's first cell and vice versa); those two known terms are
    subtracted on the host (`_BOUND_STARTS`). Core slices start at line
    starts, so taps at core edges only read the DRAM zero guards."""
    colmap = np.full((4, 256, H * W), -1, np.int64)
    starts = []
    core, col = 0, 0
    for d in range(4):
        lines = _build_lines(d)
        for b in range(256):
            for ln in lines:
                ll = len(ln)
                if col + ll > CORE_COLS:
                    core += 1
                    col = 0
                    assert core < NCORE, "stream overflow"
                if col > 0:
                    starts.append(core * CORE_COLS + col)
                for i, (r, c) in enumerate(ln):
                    colmap[d, b, r * W + c] = core * CORE_COLS + col + i
                col += ll
    assert (colmap >= 0).all()
    return colmap, np.array(starts, np.int64)


_COLMAP, _BOUND_STARTS = _build_stream_map()


def _split_drain_waits(nc, max_waits=1):
    """Workaround for this walrus build's 'Too many sync wait commands' limit
    (1 sync wait per instruction): hoist excess sem-waits onto nop
    instructions inserted right before the instruction on the same engine.
    Sequential waits on one engine queue are equivalent to multiple waits on
    one instruction."""
    # The Tile exit drain (an InstDrain with many waits, immediately followed
    # by an all-engine barrier) may have its waits distributed across ALL
    # engines — each nop then gates that engine's barrier arrival, and the
    # chains dispatch in parallel instead of serially on one queue. For any
    # other instruction the waits must stay on its own engine.
    rr_engines = [
        mybir.EngineType.SP,
        mybir.EngineType.Pool,
        mybir.EngineType.Activation,
        mybir.EngineType.DVE,
        mybir.EngineType.PE,
    ]
    n = 0
    for fn in nc.m.functions:
        for bb in fn.blocks:
            insts = bb.instructions
            i = 0
            while i < len(insts):
                inst = insts[i]
                si = inst.sync_info
                if si is not None and si.on_wait and len(si.on_wait) > max_waits:
                    is_exit_drain = (
                        type(inst).__name__ == "InstDrain" and len(si.on_wait) > 3
                    )
                    extra = list(si.on_wait)[max_waits:]
                    si.on_wait = list(si.on_wait)[:max_waits]
                    for j, wt in enumerate(extra):
                        eng = rr_engines[j % len(rr_engines)] if is_exit_drain else inst.engine
                        nop = mybir.InstNoOp(
                            name=f"I-waitsplit-{n}",
                            engine=eng,
                            sync_info=mybir.SyncInfo(on_wait=[wt], on_update=[]),
                        )
                        nc.register_instruction(nop)
                        n += 1
                        insts.insert(i, nop)
                        i += 1
                i += 1
    return n


def build_program():
    nc = bass.Bass("TRN2", target_bir_lowering=False, debug=False, num_devices=8)
    # xin[p, eo, chunk, c]: eo=0 even stream E[c]=x[2c] (guards at NP, NP+1),
    # eo=1 odd stream O[c]=x[2c-3] i.e. odd[c-2] (guards at 0, 1). The +2
    # guard offset keeps o[j] at an even element index so d1 = e - o is
    # 4B-aligned (DVE 2x mode).
    xin = nc.dram_tensor("xin", [P, 2, 2, TOT], MM_DT, kind="ExternalInput").ap()
    # wts[p, half, kind, chunk, m]; kind: 0=W1, 1=W1+W2, 2=W2, 3=W0
    wts = nc.dram_tensor("wts", [P, 2, 4, 2, P], MM_DT, kind="ExternalInput").ap()
    # yout[p, half, eo, j]: eo=0 -> y at col 2j, eo=1 -> col 2j+1
    yout = nc.dram_tensor("yout", [P, 2, 2, NP], MM_DT, kind="ExternalOutput").ap()

    f32 = mybir.dt.float32

    with tile.TileContext(nc) as tc, ExitStack() as ctx:
        cpool = ctx.enter_context(tc.tile_pool(name="const", bufs=1))
        xpool = ctx.enter_context(tc.tile_pool(name="x", bufs=8))
        dpool = ctx.enter_context(tc.tile_pool(name="d1", bufs=3))
        gpool = ctx.enter_context(tc.tile_pool(name="d2", bufs=3))
        mpool = ctx.enter_context(tc.tile_pool(name="ms", bufs=4))
        ypool = ctx.enter_context(tc.tile_pool(name="y", bufs=4))
        ppool = ctx.enter_context(tc.tile_pool(name="ps", bufs=8, space="PSUM"))

        # Weights DMA first-thing, the two cout-halves on parallel queues
        # (scalar + gpsimd) so they don't serialize behind the x prefetch on
        # the sync queue.
        wt = cpool.tile([P, 2, 4, 2, P], MM_DT)
        nc.scalar.dma_start(wt[:, 0], wts[:, 0])
        nc.gpsimd.dma_start(wt[:, 1], wts[:, 1])

        # PE warmup: dummy matmuls with no DMA dependency, issued while the
        # first transfers are in flight, so the HAM clock-gate ramps toward
        # 2.4 GHz before the real matmuls start (cold PE runs at 1.2 GHz),
        # sized to end roughly when the first block's inputs are ready.
        warm = cpool.tile([P, P], MM_DT)
        nc.gpsimd.memset(warm[:], 0.0)
        wps = ppool.tile([P, 512], f32, tag="ps")
        for i in range(16):
            nc.tensor.matmul(
                wps[:, :P], warm[:], warm[:], start=(i == 0), stop=(i == 15)
            )
        # engine warmups: load ScalarE act tables / GPSIMD paths before the
        # steady-state loop needs them.
        warm2 = cpool.tile([P, P], f32)
        nc.scalar.copy(warm2[:], wps[:, :P])
        warm3 = cpool.tile([P, P], MM_DT)
        nc.gpsimd.tensor_sub(warm3[:], warm[:], warm[:])

        b = 0
        for u in UNITS:
            xt = xpool.tile([P, 2, 2, u + 2], MM_DT)
            nc.sync.dma_start(xt[:], xin[:, :, :, b : b + u + 2])
            # views (chunk dim kept): e[j], e[j+1], o[j-1], o[j]
            ev = xt[:, 0, :, 0:u]
            ev1 = xt[:, 0, :, 1 : u + 1]
            od0 = xt[:, 1, :, 1 : u + 1]
            od = xt[:, 1, :, 2 : u + 2]
            d1 = dpool.tile([P, 2, u], MM_DT)
            nc.vector.tensor_sub(d1[:], ev, od)      # 4B-aligned -> 2x mode
            d2 = gpool.tile([P, 2, u], MM_DT)
            nc.gpsimd.tensor_sub(d2[:], ev1, od)
            yt = ypool.tile([P, 2, 2, u], MM_DT)

            small = u <= 128
            ms = [None, None]
            # A and M groups for both halves first; B groups later so the
            # (slower) GPSIMD d2 stream is ready when the PE reaches them.
            for o in range(2):
                A = ppool.tile([P, 512], f32, tag="ps")
                nc.tensor.matmul(A[:, :u], wt[:, o, 0, 0, :], d1[:, 0, :], start=True, stop=False)
                nc.tensor.matmul(A[:, :u], wt[:, o, 0, 1, :], d1[:, 1, :], start=False, stop=False)
                nc.tensor.matmul(A[:, :u], wt[:, o, 3, 0, :], xt[:, 1, 0, 1 : u + 1], start=False, stop=False)
                nc.tensor.matmul(A[:, :u], wt[:, o, 3, 1, :], xt[:, 1, 1, 1 : u + 1], start=False, stop=True)
                M = ppool.tile([P, 512], f32, tag="ps")
                nc.tensor.matmul(M[:, :u], wt[:, o, 1, 0, :], xt[:, 1, 0, 2 : u + 2], start=True, stop=False)
                nc.tensor.matmul(M[:, :u], wt[:, o, 1, 1, :], xt[:, 1, 1, 2 : u + 2], start=False, stop=True)
                # TT cannot read two PSUM operands (hardware: single PSUM
                # read port), so M is staged through SBUF via ScalarE.
                Ms = mpool.tile([P, 512], f32)
                nc.scalar.copy(Ms[:, :u], M[:, :u])
                nc.vector.tensor_add(yt[:, o, 0, :], A[:, :u], Ms[:, :u])
                ms[o] = Ms
            for o in range(2):
                B = ppool.tile([P, 512], f32, tag="ps")
                nc.tensor.matmul(B[:, :u], wt[:, o, 2, 0, :], d2[:, 0, :], start=True, stop=False)
                nc.tensor.matmul(B[:, :u], wt[:, o, 2, 1, :], d2[:, 1, :], start=False, stop=False)
                nc.tensor.matmul(B[:, :u], wt[:, o, 3, 0, :], xt[:, 0, 0, 0:u], start=False, stop=False)
                nc.tensor.matmul(B[:, :u], wt[:, o, 3, 1, :], xt[:, 0, 1, 0:u], start=False, stop=True)
                nc.vector.tensor_add(yt[:, o, 1, :], B[:, :u], ms[o][:, :u])

            if small:
                # tail/prologue: ship each cout-half as soon as its adds
                # land, on alternating DMA queues
                nc.scalar.dma_start(yout[:, 0, :, b : b + u], yt[:, 0])
                nc.sync.dma_start(yout[:, 1, :, b : b + u], yt[:, 1])
            else:
                # output rides the ScalarE queue so input (sync queue) and
                # output transfers overlap
                nc.scalar.dma_start(yout[:, :, :, b : b + u], yt[:])
            b += u
    _split_drain_waits(nc)
    return nc


def pack_inputs(xs, weight):
    """xs: list of 4 arrays [256, 256, 15, 15] fp32. in_maps for cores 0-7."""
    # kinds: 0=W1, 1=W1+W2, 2=W2, 3=W0 ; shape [4, C_out, C_in] fp32
    kinds = np.stack(
        [
            weight[:, :, 1],
            weight[:, :, 1] + weight[:, :, 2],
            weight[:, :, 2],
            weight[:, :, 0],
        ],
        axis=0,
    )
    # w_dev[p, half, kind, chunk, m] = kinds[kind, half*128+m, chunk*128+p]
    w_dev = np.ascontiguousarray(
        kinds.reshape(4, 2, P, 2, P)      # [kind, half, m, chunk, p]
        .transpose(4, 1, 0, 3, 2)         # [p, half, kind, chunk, m]
    ).astype(MM_NP)

    C = 256
    xflat = np.zeros((C, NCORE * CORE_COLS), MM_NP)
    for d in range(4):
        xflat[:, _COLMAP[d].reshape(-1)] = (
            xs[d].transpose(1, 0, 2, 3).reshape(C, -1).astype(MM_NP)
        )

    in_maps = []
    for core in range(NCORE):
        seg = xflat[:, core * CORE_COLS : (core + 1) * CORE_COLS]
        ev = seg[:, 0::2]   # [256, NP]
        od = seg[:, 1::2]
        xin_np = np.zeros((P, 2, 2, TOT), MM_NP)
        xin_np[:, 0, 0, :NP] = ev[:P]
        xin_np[:, 0, 1, :NP] = ev[P:]
        xin_np[:, 1, 0, 2:] = od[:P]
        xin_np[:, 1, 1, 2:] = od[P:]
        in_maps.append({"xin": xin_np, "wts": w_dev})
    return in_maps, xflat


def unpack_outputs(results, bias, weight, xflat):
    O = 256
    yflat = np.empty((O, NCORE * CORE_COLS), np.float32)
    for core in range(NCORE):
        yo = np.asarray(results[core]["yout"])      # [128, 2, 2, NP] fp16
        seg = (
            yo.transpose(1, 0, 3, 2)                # [half, p, j, eo]
            .reshape(O, CORE_COLS)
            .astype(np.float32)
        )
        yflat[:, core * CORE_COLS : (core + 1) * CORE_COLS] = seg

    # subtract line-junction contamination (fp16-quantized operands match
    # what the device multiplied, so the residual is only PSUM rounding)
    starts = _BOUND_STARTS
    ends = starts - 1
    w0q = weight[:, :, 0].astype(MM_NP).astype(np.float32)
    w2q = weight[:, :, 2].astype(MM_NP).astype(np.float32)
    xs_q = xflat[:, starts].astype(np.float32)
    xe_q = xflat[:, ends].astype(np.float32)
    yflat[:, ends] -= w2q @ xs_q
    yflat[:, starts] -= w0q @ xe_q
    outs = []
    b = bias[None, :, None].astype(np.float32)
    for d in range(4):
        yd = yflat[:, _COLMAP[d].reshape(-1)].reshape(O, 256, H * W)
        yd = yd.transpose(1, 0, 2) + b
        outs.append(np.ascontiguousarray(yd.reshape(256, 256, H, W)))
    return tuple(outs)


def kernel(x0, x1, x2, x3, weight, bias):
    xs = [np.ascontiguousarray(np.asarray(a, dtype=np.float32)) for a in (x0, x1, x2, x3)]
    weight = np.asarray(weight, dtype=np.float32)
    bias = np.asarray(bias, dtype=np.float32)

    nc = build_program()
    in_maps, xflat = pack_inputs(xs, weight)
    res = run_bass_kernel_spmd(nc, in_maps, list(range(NCORE)))
    return unpack_outputs(res.results, bias, weight, xflat)


# revision 15
# speedup vs baseline: 1.1917x; 1.1917x over previous
"""Trainium2 Bass kernel for DirectionalConvLayer.

Problem: 4 directional 3-tap convs over [256, 256, 15, 15] fp32 images, one
input per direction (horizontal / vertical / main-diagonal / anti-diagonal
taps), shared weight [256, 256, 3] and bias [256].

Strategy: every direction is a 1-D 3-tap conv along its set of lines
(rows / columns / diagonals / anti-diagonals) with a dense 256x256 channel
mix per tap. On the host, ALL lines of ALL four inputs are packed
back-to-back (no separators) into one flat stream, split across 8 cores at
line boundaries. The device kernel is direction-agnostic: a pure 3-tap conv
along the flat axis. The conv contaminates the two outputs at every line
junction with one known term each; the host subtracts those (two batched
matmuls) during unpacking.

PE-work reduction via partial Winograd F(2,2): for an output pair
(y_e, y_o) at stream cols (2j, 2j+1), with e[j] = x[2j], o[j] = x[2j+1]:
  y_e = W0 o[j-1] + W1 e[j] + W2 o[j]
  y_o = W0 e[j]   + W1 o[j] + W2 e[j+1]
Winograd F(2,2) on the (W1, W2) 2-tap part shares the midpoint product
M = (W1+W2) o[j]:
  y_e = [W1 (e[j]-o[j])     + W0 o[j-1]] + M   = A + M
  y_o = [W2 (e[j+1]-o[j])   + W0 e[j]  ] + M   = B + M
=> 5 matmul products per 2 output columns instead of the naive 6 (PE time
x5/6), at the cost of 2 cheap fp16 difference streams (DVE + GPSIMD), one
PSUM->SBUF copy of M per cout-half (ScalarE), and PSUM+SBUF adds (DVE).
All transform work hides under the PE.

Transfers and matmul operands are float16 (10-bit mantissa); PSUM
accumulates in fp32. Host packs even/odd deinterleaved streams (guard
offsets chosen so the d1 subtraction is 4B-aligned -> DVE 2x mode); host
adds bias and fixes line junctions during unpacking.
"""
from contextlib import ExitStack

import numpy as np

import concourse.bass as bass
import concourse.tile as tile
from concourse import mybir
from concourse.bass_utils import run_bass_kernel_spmd

P = 128
H = W = 15
NCORE = 8
CORE_COLS = 28808          # stream cols per core (must be even)
NP = CORE_COLS // 2        # 14404 output pairs per core
TOT = NP + 2               # packed stream length incl. guards

MM_DT = mybir.dt.float16
MM_NP = np.float16

# DMA units (lists of compute sub-block pair counts): small prologue units
# for fast PE start, big middle units to amortize the ~700ns per-DMA
# descriptor-generation cost on the queue, small tail units to ship the
# last outputs early. All sub-block sizes and offsets even (4B alignment
# of the d1 subtraction views).
UNITS = [[128], [128], [256]] + [[512, 512]] * 13 + [[256], [160], [100], [64]]
assert sum(sum(u) for u in UNITS) == NP
assert all(s % 2 == 0 for u in UNITS for s in u)


def _build_lines(d):
    if d == 0:
        return [[(i, j) for j in range(W)] for i in range(H)]
    if d == 1:
        return [[(i, j) for i in range(H)] for j in range(W)]
    if d == 2:
        return [
            [(i, i - k) for i in range(max(0, k), min(H, H + k))]
            for k in range(-(W - 1), W)
        ]
    return [
        [(i, s - i) for i in range(max(0, s - (W - 1)), min(H, s + 1))]
        for s in range(H + W - 1)
    ]


def _build_stream_map():
    """Greedy-pack every (direction, image, line) into NCORE x CORE_COLS,
    back-to-back with NO separators. colmap[d, b, i*W+j] = core * CORE_COLS
    + local_col. At every line-to-line junction the device conv contaminates
    the two adjacent outputs (tap w2 of the left line's last cell reads the
    right line's first cell and vice versa); those two known terms are
    subtracted on the host (`_BOUND_STARTS`). Core slices start at line
    starts, so taps at core edges only read the DRAM zero guards."""
    colmap = np.full((4, 256, H * W), -1, np.int64)
    starts = []
    core, col = 0, 0
    for d in range(4):
        lines = _build_lines(d)
        for b in range(256):
            for ln in lines:
                ll = len(ln)
                if col + ll > CORE_COLS:
                    core += 1
                    col = 0
                    assert core < NCORE, "stream overflow"
                if col > 0:
                    starts.append(core * CORE_COLS + col)
                for i, (r, c) in enumerate(ln):
                    colmap[d, b, r * W + c] = core * CORE_COLS + col + i
                col += ll
    assert (colmap >= 0).all()
    return colmap, np.array(starts, np.int64)


_COLMAP, _BOUND_STARTS = _build_stream_map()


def _split_drain_waits(nc, max_waits=1):
    """Workaround for this walrus build's 'Too many sync wait commands' limit
    (1 sync wait per instruction): hoist excess sem-waits onto nop
    instructions inserted right before the instruction on the same engine.
    Sequential waits on one engine queue are equivalent to multiple waits on
    one instruction."""
    # The Tile exit drain (an InstDrain with many waits, immediately followed
    # by an all-engine barrier) may have its waits distributed across ALL
    # engines — each nop then gates that engine's barrier arrival, and the
    # chains dispatch in parallel instead of serially on one queue. For any
    # other instruction the waits must stay on its own engine.
    rr_engines = [
        mybir.EngineType.SP,
        mybir.EngineType.Pool,
        mybir.EngineType.Activation,
        mybir.EngineType.DVE,
        mybir.EngineType.PE,
    ]
    n = 0
    for fn in nc.m.functions:
        for bb in fn.blocks:
            insts = bb.instructions
            i = 0
            while i < len(insts):
                inst = insts[i]
                si = inst.sync_info
                if si is not None and si.on_wait and len(si.on_wait) > max_waits:
                    is_exit_drain = (
                        type(inst).__name__ == "InstDrain" and len(si.on_wait) > 3
                    )
                    extra = list(si.on_wait)[max_waits:]
                    si.on_wait = list(si.on_wait)[:max_waits]
                    for j, wt in enumerate(extra):
                        eng = rr_engines[j % len(rr_engines)] if is_exit_drain else inst.engine
                        nop = mybir.InstNoOp(
                            name=f"I-waitsplit-{n}",
                            engine=eng,
                            sync_info=mybir.SyncInfo(on_wait=[wt], on_update=[]),
                        )
                        nc.register_instruction(nop)
                        n += 1
                        insts.insert(i, nop)
                        i += 1
                i += 1
    return n


def build_program():
    nc = bass.Bass("TRN2", target_bir_lowering=False, debug=False, num_devices=8)
    # xin[p, eo, chunk, c]: eo=0 even stream E[c]=x[2c] (guards at NP, NP+1),
    # eo=1 odd stream O[c]=x[2c-3] i.e. odd[c-2] (guards at 0, 1). The +2
    # guard offset keeps o[j] at an even element index so d1 = e - o is
    # 4B-aligned (DVE 2x mode).
    xin = nc.dram_tensor("xin", [P, 2, 2, TOT], MM_DT, kind="ExternalInput").ap()
    # wts[p, half, kind, chunk, m]; kind: 0=W1, 1=W1+W2, 2=W2, 3=W0
    wts = nc.dram_tensor("wts", [P, 2, 4, 2, P], MM_DT, kind="ExternalInput").ap()
    # yout[p, half, eo, j]: eo=0 -> y at col 2j, eo=1 -> col 2j+1
    yout = nc.dram_tensor("yout", [P, 2, 2, NP], MM_DT, kind="ExternalOutput").ap()

    f32 = mybir.dt.float32

    with tile.TileContext(nc) as tc, ExitStack() as ctx:
        cpool = ctx.enter_context(tc.tile_pool(name="const", bufs=1))
        xpool = ctx.enter_context(tc.tile_pool(name="x", bufs=8))
        dpool = ctx.enter_context(tc.tile_pool(name="d1", bufs=3))
        gpool = ctx.enter_context(tc.tile_pool(name="d2", bufs=3))
        mpool = ctx.enter_context(tc.tile_pool(name="ms", bufs=4))
        ypool = ctx.enter_context(tc.tile_pool(name="y", bufs=4))
        ppool = ctx.enter_context(tc.tile_pool(name="ps", bufs=8, space="PSUM"))

        # Weights DMA first-thing, the two cout-halves on parallel queues
        # (scalar + gpsimd) so they don't serialize behind the x prefetch on
        # the sync queue.
        wt = cpool.tile([P, 2, 4, 2, P], MM_DT)
        nc.scalar.dma_start(wt[:, 0], wts[:, 0])
        nc.gpsimd.dma_start(wt[:, 1], wts[:, 1])

        # PE warmup: dummy matmuls with no DMA dependency, issued while the
        # first transfers are in flight, so the HAM clock-gate ramps toward
        # 2.4 GHz before the real matmuls start (cold PE runs at 1.2 GHz),
        # sized to end roughly when the first block's inputs are ready.
        warm = cpool.tile([P, P], MM_DT)
        nc.gpsimd.memset(warm[:], 0.0)
        wps = ppool.tile([P, 512], f32, tag="ps")
        for i in range(16):
            nc.tensor.matmul(
                wps[:, :P], warm[:], warm[:], start=(i == 0), stop=(i == 15)
            )
        # engine warmups: load ScalarE act tables / GPSIMD paths before the
        # steady-state loop needs them.
        warm2 = cpool.tile([P, P], f32)
        nc.scalar.copy(warm2[:], wps[:, :P])
        warm3 = cpool.tile([P, P], MM_DT)
        nc.gpsimd.tensor_sub(warm3[:], warm[:], warm[:])

        b = 0
        for unit in UNITS:
            utot = sum(unit)
            xt = xpool.tile([P, 2, 2, utot + 2], MM_DT)
            nc.sync.dma_start(xt[:], xin[:, :, :, b : b + utot + 2])
            yt = ypool.tile([P, 2, 2, utot], MM_DT)
            cb = 0
            for u in unit:
                # views (chunk dim kept): e[j], e[j+1], o[j-1], o[j]
                ev = xt[:, 0, :, cb : cb + u]
                ev1 = xt[:, 0, :, cb + 1 : cb + u + 1]
                od = xt[:, 1, :, cb + 2 : cb + u + 2]
                d1 = dpool.tile([P, 2, u], MM_DT)
                nc.vector.tensor_sub(d1[:], ev, od)   # 4B-aligned -> 2x mode
                d2 = gpool.tile([P, 2, u], MM_DT)
                nc.gpsimd.tensor_sub(d2[:], ev1, od)

                ms = [None, None]
                # A and M groups for both halves first; B groups later so
                # the (slower) GPSIMD d2 stream is ready when the PE reaches
                # them.
                for o in range(2):
                    A = ppool.tile([P, 512], f32, tag="ps")
                    nc.tensor.matmul(A[:, :u], wt[:, o, 0, 0, :], d1[:, 0, :], start=True, stop=False)
                    nc.tensor.matmul(A[:, :u], wt[:, o, 0, 1, :], d1[:, 1, :], start=False, stop=False)
                    nc.tensor.matmul(A[:, :u], wt[:, o, 3, 0, :], xt[:, 1, 0, cb + 1 : cb + u + 1], start=False, stop=False)
                    nc.tensor.matmul(A[:, :u], wt[:, o, 3, 1, :], xt[:, 1, 1, cb + 1 : cb + u + 1], start=False, stop=True)
                    M = ppool.tile([P, 512], f32, tag="ps")
                    nc.tensor.matmul(M[:, :u], wt[:, o, 1, 0, :], xt[:, 1, 0, cb + 2 : cb + u + 2], start=True, stop=False)
                    nc.tensor.matmul(M[:, :u], wt[:, o, 1, 1, :], xt[:, 1, 1, cb + 2 : cb + u + 2], start=False, stop=True)
                    # TT cannot read two PSUM operands (hardware: single
                    # PSUM read port), so M is staged through SBUF.
                    Ms = mpool.tile([P, 512], f32)
                    nc.scalar.copy(Ms[:, :u], M[:, :u])
                    nc.vector.tensor_add(yt[:, o, 0, cb : cb + u], A[:, :u], Ms[:, :u])
                    ms[o] = Ms
                for o in range(2):
                    B = ppool.tile([P, 512], f32, tag="ps")
                    nc.tensor.matmul(B[:, :u], wt[:, o, 2, 0, :], d2[:, 0, :], start=True, stop=False)
                    nc.tensor.matmul(B[:, :u], wt[:, o, 2, 1, :], d2[:, 1, :], start=False, stop=False)
                    nc.tensor.matmul(B[:, :u], wt[:, o, 3, 0, :], xt[:, 0, 0, cb : cb + u], start=False, stop=False)
                    nc.tensor.matmul(B[:, :u], wt[:, o, 3, 1, :], xt[:, 0, 1, cb : cb + u], start=False, stop=True)
                    nc.vector.tensor_add(yt[:, o, 1, cb : cb + u], B[:, :u], ms[o][:, :u])
                cb += u

            if utot <= 128:
                # tail/prologue: ship each cout-half as soon as its adds
                # land, on alternating DMA queues
                nc.scalar.dma_start(yout[:, 0, :, b : b + utot], yt[:, 0])
                nc.sync.dma_start(yout[:, 1, :, b : b + utot], yt[:, 1])
            else:
                # output rides the ScalarE queue so input (sync queue) and
                # output transfers overlap
                nc.scalar.dma_start(yout[:, :, :, b : b + utot], yt[:])
            b += utot
    _split_drain_waits(nc)
    return nc


def pack_inputs(xs, weight):
    """xs: list of 4 arrays [256, 256, 15, 15] fp32. in_maps for cores 0-7."""
    # kinds: 0=W1, 1=W1+W2, 2=W2, 3=W0 ; shape [4, C_out, C_in] fp32
    kinds = np.stack(
        [
            weight[:, :, 1],
            weight[:, :, 1] + weight[:, :, 2],
            weight[:, :, 2],
            weight[:, :, 0],
        ],
        axis=0,
    )
    # w_dev[p, half, kind, chunk, m] = kinds[kind, half*128+m, chunk*128+p]
    w_dev = np.ascontiguousarray(
        kinds.reshape(4, 2, P, 2, P)      # [kind, half, m, chunk, p]
        .transpose(4, 1, 0, 3, 2)         # [p, half, kind, chunk, m]
    ).astype(MM_NP)

    C = 256
    xflat = np.zeros((C, NCORE * CORE_COLS), MM_NP)
    for d in range(4):
        xflat[:, _COLMAP[d].reshape(-1)] = (
            xs[d].transpose(1, 0, 2, 3).reshape(C, -1).astype(MM_NP)
        )

    in_maps = []
    for core in range(NCORE):
        seg = xflat[:, core * CORE_COLS : (core + 1) * CORE_COLS]
        ev = seg[:, 0::2]   # [256, NP]
        od = seg[:, 1::2]
        xin_np = np.zeros((P, 2, 2, TOT), MM_NP)
        xin_np[:, 0, 0, :NP] = ev[:P]
        xin_np[:, 0, 1, :NP] = ev[P:]
        xin_np[:, 1, 0, 2:] = od[:P]
        xin_np[:, 1, 1, 2:] = od[P:]
        in_maps.append({"xin": xin_np, "wts": w_dev})
    return in_maps, xflat


def unpack_outputs(results, bias, weight, xflat):
    O = 256
    yflat = np.empty((O, NCORE * CORE_COLS), np.float32)
    for core in range(NCORE):
        yo = np.asarray(results[core]["yout"])      # [128, 2, 2, NP] fp16
        seg = (
            yo.transpose(1, 0, 3, 2)                # [half, p, j, eo]
            .reshape(O, CORE_COLS)
            .astype(np.float32)
        )
        yflat[:, core * CORE_COLS : (core + 1) * CORE_COLS] = seg

    # subtract line-junction contamination (fp16-quantized operands match
    # what the device multiplied, so the residual is only PSUM rounding)
    starts = _BOUND_STARTS
    ends = starts - 1
    w0q = weight[:, :, 0].astype(MM_NP).astype(np.float32)
    w2q = weight[:, :, 2].astype(MM_NP).astype(np.float32)
    xs_q = xflat[:, starts].astype(np.float32)
    xe_q = xflat[:, ends].astype(np.float32)
    yflat[:, ends] -= w2q @ xs_q
    yflat[:, starts] -= w0q @ xe_q
    outs = []
    b = bias[None, :, None].astype(np.float32)
    for d in range(4):
        yd = yflat[:, _COLMAP[d].reshape(-1)].reshape(O, 256, H * W)
        yd = yd.transpose(1, 0, 2) + b
        outs.append(np.ascontiguousarray(yd.reshape(256, 256, H, W)))
    return tuple(outs)


def kernel(x0, x1, x2, x3, weight, bias):
    xs = [np.ascontiguousarray(np.asarray(a, dtype=np.float32)) for a in (x0, x1, x2, x3)]
    weight = np.asarray(weight, dtype=np.float32)
    bias = np.asarray(bias, dtype=np.float32)

    nc = build_program()
    in_maps, xflat = pack_inputs(xs, weight)
    res = run_bass_kernel_spmd(nc, in_maps, list(range(NCORE)))
    return unpack_outputs(res.results, bias, weight, xflat)


# revision 17
# speedup vs baseline: 1.2029x; 1.0094x over previous
"""Trainium2 Bass kernel for DirectionalConvLayer.

Problem: 4 directional 3-tap convs over [256, 256, 15, 15] fp32 images, one
input per direction (horizontal / vertical / main-diagonal / anti-diagonal
taps), shared weight [256, 256, 3] and bias [256].

Strategy: every direction is a 1-D 3-tap conv along its set of lines
(rows / columns / diagonals / anti-diagonals) with a dense 256x256 channel
mix per tap. On the host, ALL lines of ALL four inputs are packed
back-to-back (no separators) into one flat stream, split across 8 cores at
line boundaries. The device kernel is direction-agnostic: a pure 3-tap conv
along the flat axis. The conv contaminates the two outputs at every line
junction with one known term each; the host subtracts those (two batched
matmuls) during unpacking.

PE-work reduction via partial Winograd F(2,2): for an output pair
(y_e, y_o) at stream cols (2j, 2j+1), with e[j] = x[2j], o[j] = x[2j+1]:
  y_e = W0 o[j-1] + W1 e[j] + W2 o[j]
  y_o = W0 e[j]   + W1 o[j] + W2 e[j+1]
Winograd F(2,2) on the (W1, W2) 2-tap part shares the midpoint product
M = (W1+W2) o[j]:
  y_e = [W1 (e[j]-o[j])     + W0 o[j-1]] + M   = A + M
  y_o = [W2 (e[j+1]-o[j])   + W0 e[j]  ] + M   = B + M
=> 5 matmul products per 2 output columns instead of the naive 6 (PE time
x5/6), at the cost of 2 cheap fp16 difference streams (DVE + GPSIMD), one
PSUM->SBUF copy of M per cout-half (ScalarE), and PSUM+SBUF adds (DVE).
All transform work hides under the PE.

Transfers and matmul operands are float16 (10-bit mantissa); PSUM
accumulates in fp32. Host packs even/odd deinterleaved streams (guard
offsets chosen so the d1 subtraction is 4B-aligned -> DVE 2x mode); host
adds bias and fixes line junctions during unpacking.
"""
from contextlib import ExitStack

import numpy as np

import concourse.bass as bass
import concourse.tile as tile
from concourse import mybir
from concourse.bass_utils import run_bass_kernel_spmd

P = 128
H = W = 15
NCORE = 8
CORE_COLS = 28808          # stream cols per core (must be even)
NP = CORE_COLS // 2        # 14404 output pairs per core
TOT = NP + 2               # packed stream length incl. guards

MM_DT = mybir.dt.float16
MM_NP = np.float16

# DMA units (lists of compute sub-block pair counts): small prologue units
# for fast PE start, big middle units to amortize the ~700ns per-DMA
# descriptor-generation cost on the queue, small tail units to ship the
# last outputs early. All sub-block sizes and offsets even (4B alignment
# of the d1 subtraction views).
UNITS = (
    [[256], [512]]
    + [[512, 512]] * 12
    + [[512], [256]]
    + [[256], [160], [100], [64]]
)
assert sum(sum(u) for u in UNITS) == NP
assert all(s % 2 == 0 for u in UNITS for s in u)


def _build_lines(d):
    if d == 0:
        return [[(i, j) for j in range(W)] for i in range(H)]
    if d == 1:
        return [[(i, j) for i in range(H)] for j in range(W)]
    if d == 2:
        return [
            [(i, i - k) for i in range(max(0, k), min(H, H + k))]
            for k in range(-(W - 1), W)
        ]
    return [
        [(i, s - i) for i in range(max(0, s - (W - 1)), min(H, s + 1))]
        for s in range(H + W - 1)
    ]


def _build_stream_map():
    """Greedy-pack every (direction, image, line) into NCORE x CORE_COLS,
    back-to-back with NO separators. colmap[d, b, i*W+j] = core * CORE_COLS
    + local_col. At every line-to-line junction the device conv contaminates
    the two adjacent outputs (tap w2 of the left line's last cell reads the
    right line's first cell and vice versa); those two known terms are
    subtracted on the host (`_BOUND_STARTS`). Core slices start at line
    starts, so taps at core edges only read the DRAM zero guards."""
    colmap = np.full((4, 256, H * W), -1, np.int64)
    starts = []
    core, col = 0, 0
    for d in range(4):
        lines = _build_lines(d)
        for b in range(256):
            for ln in lines:
                ll = len(ln)
                if col + ll > CORE_COLS:
                    core += 1
                    col = 0
                    assert core < NCORE, "stream overflow"
                if col > 0:
                    starts.append(core * CORE_COLS + col)
                for i, (r, c) in enumerate(ln):
                    colmap[d, b, r * W + c] = core * CORE_COLS + col + i
                col += ll
    assert (colmap >= 0).all()
    return colmap, np.array(starts, np.int64)


_COLMAP, _BOUND_STARTS = _build_stream_map()


def _split_drain_waits(nc, max_waits=1):
    """Workaround for this walrus build's 'Too many sync wait commands' limit
    (1 sync wait per instruction): hoist excess sem-waits onto nop
    instructions inserted right before the instruction on the same engine.
    Sequential waits on one engine queue are equivalent to multiple waits on
    one instruction."""
    # The Tile exit drain (an InstDrain with many waits, immediately followed
    # by an all-engine barrier) may have its waits distributed across ALL
    # engines — each nop then gates that engine's barrier arrival, and the
    # chains dispatch in parallel instead of serially on one queue. For any
    # other instruction the waits must stay on its own engine.
    rr_engines = [
        mybir.EngineType.SP,
        mybir.EngineType.Pool,
        mybir.EngineType.Activation,
        mybir.EngineType.DVE,
        mybir.EngineType.PE,
    ]
    n = 0
    for fn in nc.m.functions:
        for bb in fn.blocks:
            insts = bb.instructions
            i = 0
            while i < len(insts):
                inst = insts[i]
                si = inst.sync_info
                if si is not None and si.on_wait and len(si.on_wait) > max_waits:
                    is_exit_drain = (
                        type(inst).__name__ == "InstDrain" and len(si.on_wait) > 3
                    )
                    extra = list(si.on_wait)[max_waits:]
                    si.on_wait = list(si.on_wait)[:max_waits]
                    for j, wt in enumerate(extra):
                        eng = rr_engines[j % len(rr_engines)] if is_exit_drain else inst.engine
                        nop = mybir.InstNoOp(
                            name=f"I-waitsplit-{n}",
                            engine=eng,
                            sync_info=mybir.SyncInfo(on_wait=[wt], on_update=[]),
                        )
                        nc.register_instruction(nop)
                        n += 1
                        insts.insert(i, nop)
                        i += 1
                i += 1
    return n


def build_program():
    nc = bass.Bass("TRN2", target_bir_lowering=False, debug=False, num_devices=8)
    # xin[p, eo, chunk, c]: eo=0 even stream E[c]=x[2c] (guards at NP, NP+1),
    # eo=1 odd stream O[c]=x[2c-3] i.e. odd[c-2] (guards at 0, 1). The +2
    # guard offset keeps o[j] at an even element index so d1 = e - o is
    # 4B-aligned (DVE 2x mode).
    xin = nc.dram_tensor("xin", [P, 2, 2, TOT], MM_DT, kind="ExternalInput").ap()
    # wts[p, half, kind, chunk, m]; kind: 0=W1, 1=W1+W2, 2=W2, 3=W0
    wts = nc.dram_tensor("wts", [P, 2, 4, 2, P], MM_DT, kind="ExternalInput").ap()
    # yout[p, half, eo, j]: eo=0 -> y at col 2j, eo=1 -> col 2j+1
    yout = nc.dram_tensor("yout", [P, 2, 2, NP], MM_DT, kind="ExternalOutput").ap()

    f32 = mybir.dt.float32

    with tile.TileContext(nc) as tc, ExitStack() as ctx:
        cpool = ctx.enter_context(tc.tile_pool(name="const", bufs=1))
        xpool = ctx.enter_context(tc.tile_pool(name="x", bufs=8))
        dpool = ctx.enter_context(tc.tile_pool(name="d1", bufs=3))
        gpool = ctx.enter_context(tc.tile_pool(name="d2", bufs=3))
        mpool = ctx.enter_context(tc.tile_pool(name="ms", bufs=4))
        ypool = ctx.enter_context(tc.tile_pool(name="y", bufs=4))
        ppool = ctx.enter_context(tc.tile_pool(name="ps", bufs=8, space="PSUM"))

        # Weights DMA first-thing, the two cout-halves on parallel queues
        # (scalar + gpsimd) so they don't serialize behind the x prefetch on
        # the sync queue.
        wt = cpool.tile([P, 2, 4, 2, P], MM_DT)
        nc.scalar.dma_start(wt[:, 0], wts[:, 0])
        nc.gpsimd.dma_start(wt[:, 1], wts[:, 1])

        # PE warmup: dummy matmuls with no DMA dependency, issued while the
        # first transfers are in flight, so the HAM clock-gate ramps toward
        # 2.4 GHz before the real matmuls start (cold PE runs at 1.2 GHz),
        # sized to end roughly when the first block's inputs are ready.
        warm = cpool.tile([P, P], MM_DT)
        nc.gpsimd.memset(warm[:], 0.0)
        wps = ppool.tile([P, 512], f32, tag="ps")
        for i in range(26):
            nc.tensor.matmul(
                wps[:, :P], warm[:], warm[:], start=(i == 0), stop=(i == 25)
            )
        # engine warmups: load ScalarE act tables / GPSIMD paths before the
        # steady-state loop needs them.
        warm2 = cpool.tile([P, P], f32)
        nc.scalar.copy(warm2[:], wps[:, :P])
        warm3 = cpool.tile([P, P], MM_DT)
        nc.gpsimd.tensor_sub(warm3[:], warm[:], warm[:])

        b = 0
        for unit in UNITS:
            utot = sum(unit)
            xt = xpool.tile([P, 2, 2, utot + 2], MM_DT)
            nc.sync.dma_start(xt[:], xin[:, :, :, b : b + utot + 2])
            yt = ypool.tile([P, 2, 2, utot], MM_DT)
            cb = 0
            for u in unit:
                # views (chunk dim kept): e[j], e[j+1], o[j-1], o[j]
                ev = xt[:, 0, :, cb : cb + u]
                ev1 = xt[:, 0, :, cb + 1 : cb + u + 1]
                od = xt[:, 1, :, cb + 2 : cb + u + 2]
                d1 = dpool.tile([P, 2, u], MM_DT)
                nc.vector.tensor_sub(d1[:], ev, od)   # 4B-aligned -> 2x mode
                d2 = gpool.tile([P, 2, u], MM_DT)
                nc.gpsimd.tensor_sub(d2[:], ev1, od)

                ms = [None, None]
                # A and M groups for both halves first; B groups later so
                # the (slower) GPSIMD d2 stream is ready when the PE reaches
                # them.
                for o in range(2):
                    A = ppool.tile([P, 512], f32, tag="ps")
                    nc.tensor.matmul(A[:, :u], wt[:, o, 0, 0, :], d1[:, 0, :], start=True, stop=False)
                    nc.tensor.matmul(A[:, :u], wt[:, o, 0, 1, :], d1[:, 1, :], start=False, stop=False)
                    nc.tensor.matmul(A[:, :u], wt[:, o, 3, 0, :], xt[:, 1, 0, cb + 1 : cb + u + 1], start=False, stop=False)
                    nc.tensor.matmul(A[:, :u], wt[:, o, 3, 1, :], xt[:, 1, 1, cb + 1 : cb + u + 1], start=False, stop=True)
                    M = ppool.tile([P, 512], f32, tag="ps")
                    nc.tensor.matmul(M[:, :u], wt[:, o, 1, 0, :], xt[:, 1, 0, cb + 2 : cb + u + 2], start=True, stop=False)
                    nc.tensor.matmul(M[:, :u], wt[:, o, 1, 1, :], xt[:, 1, 1, cb + 2 : cb + u + 2], start=False, stop=True)
                    # TT cannot read two PSUM operands (hardware: single
                    # PSUM read port), so M is staged through SBUF.
                    Ms = mpool.tile([P, 512], f32)
                    nc.scalar.copy(Ms[:, :u], M[:, :u])
                    nc.vector.tensor_add(yt[:, o, 0, cb : cb + u], A[:, :u], Ms[:, :u])
                    ms[o] = Ms
                for o in range(2):
                    B = ppool.tile([P, 512], f32, tag="ps")
                    nc.tensor.matmul(B[:, :u], wt[:, o, 2, 0, :], d2[:, 0, :], start=True, stop=False)
                    nc.tensor.matmul(B[:, :u], wt[:, o, 2, 1, :], d2[:, 1, :], start=False, stop=False)
                    nc.tensor.matmul(B[:, :u], wt[:, o, 3, 0, :], xt[:, 0, 0, cb : cb + u], start=False, stop=False)
                    nc.tensor.matmul(B[:, :u], wt[:, o, 3, 1, :], xt[:, 0, 1, cb : cb + u], start=False, stop=True)
                    nc.vector.tensor_add(yt[:, o, 1, cb : cb + u], B[:, :u], ms[o][:, :u])
                cb += u

            if utot <= 128:
                # tail/prologue: ship each cout-half as soon as its adds
                # land, on alternating DMA queues
                nc.scalar.dma_start(yout[:, 0, :, b : b + utot], yt[:, 0])
                nc.sync.dma_start(yout[:, 1, :, b : b + utot], yt[:, 1])
            else:
                # output rides the ScalarE queue so input (sync queue) and
                # output transfers overlap
                nc.scalar.dma_start(yout[:, :, :, b : b + utot], yt[:])
            b += utot
    _split_drain_waits(nc)
    return nc


def pack_inputs(xs, weight):
    """xs: list of 4 arrays [256, 256, 15, 15] fp32. in_maps for cores 0-7."""
    # kinds: 0=W1, 1=W1+W2, 2=W2, 3=W0 ; shape [4, C_out, C_in] fp32
    kinds = np.stack(
        [
            weight[:, :, 1],
            weight[:, :, 1] + weight[:, :, 2],
            weight[:, :, 2],
            weight[:, :, 0],
        ],
        axis=0,
    )
    # w_dev[p, half, kind, chunk, m] = kinds[kind, half*128+m, chunk*128+p]
    w_dev = np.ascontiguousarray(
        kinds.reshape(4, 2, P, 2, P)      # [kind, half, m, chunk, p]
        .transpose(4, 1, 0, 3, 2)         # [p, half, kind, chunk, m]
    ).astype(MM_NP)

    C = 256
    xflat = np.zeros((C, NCORE * CORE_COLS), MM_NP)
    for d in range(4):
        xflat[:, _COLMAP[d].reshape(-1)] = (
            xs[d].transpose(1, 0, 2, 3).reshape(C, -1).astype(MM_NP)
        )

    in_maps = []
    for core in range(NCORE):
        seg = xflat[:, core * CORE_COLS : (core + 1) * CORE_COLS]
        ev = seg[:, 0::2]   # [256, NP]
        od = seg[:, 1::2]
        xin_np = np.zeros((P, 2, 2, TOT), MM_NP)
        xin_np[:, 0, 0, :NP] = ev[:P]
        xin_np[:, 0, 1, :NP] = ev[P:]
        xin_np[:, 1, 0, 2:] = od[:P]
        xin_np[:, 1, 1, 2:] = od[P:]
        in_maps.append({"xin": xin_np, "wts": w_dev})
    return in_maps, xflat


def unpack_outputs(results, bias, weight, xflat):
    O = 256
    yflat = np.empty((O, NCORE * CORE_COLS), np.float32)
    for core in range(NCORE):
        yo = np.asarray(results[core]["yout"])      # [128, 2, 2, NP] fp16
        seg = (
            yo.transpose(1, 0, 3, 2)                # [half, p, j, eo]
            .reshape(O, CORE_COLS)
            .astype(np.float32)
        )
        yflat[:, core * CORE_COLS : (core + 1) * CORE_COLS] = seg

    # subtract line-junction contamination (fp16-quantized operands match
    # what the device multiplied, so the residual is only PSUM rounding)
    starts = _BOUND_STARTS
    ends = starts - 1
    w0q = weight[:, :, 0].astype(MM_NP).astype(np.float32)
    w2q = weight[:, :, 2].astype(MM_NP).astype(np.float32)
    xs_q = xflat[:, starts].astype(np.float32)
    xe_q = xflat[:, ends].astype(np.float32)
    yflat[:, ends] -= w2q @ xs_q
    yflat[:, starts] -= w0q @ xe_q
    outs = []
    b = bias[None, :, None].astype(np.float32)
    for d in range(4):
        yd = yflat[:, _COLMAP[d].reshape(-1)].reshape(O, 256, H * W)
        yd = yd.transpose(1, 0, 2) + b
        outs.append(np.ascontiguousarray(yd.reshape(256, 256, H, W)))
    return tuple(outs)


def kernel(x0, x1, x2, x3, weight, bias):
    xs = [np.ascontiguousarray(np.asarray(a, dtype=np.float32)) for a in (x0, x1, x2, x3)]
    weight = np.asarray(weight, dtype=np.float32)
    bias = np.asarray(bias, dtype=np.float32)

    nc = build_program()
    in_maps, xflat = pack_inputs(xs, weight)
    res = run_bass_kernel_spmd(nc, in_maps, list(range(NCORE)))
    return unpack_outputs(res.results, bias, weight, xflat)
